# revision 6
# baseline (speedup 1.0000x reference)
"""DCGRU cell on 8 Trainium2 NeuronCores.

Sharding: data-parallel over batch (B=32 -> 4 per core), adjacency + MLP
weights replicated. No collectives; host gathers per-core outputs.

Per-core layouts (all f32):
  node-major (nm): [16 tiles][128 nodes, 768] cols = b*192+f   (diffusion lhsT)
  feat-major (fm): [6 tiles][128 bf-rows, 2048 nodes]          (hop outputs, MLP rhs)
Hop matmul: out_fm[bf, i] = sum_j x_nm[j, bf] * W[i, j]
  = matmul(lhsT=x_nm[jt][:, c*128:+128], rhs=WT[jt][:, i-block]) accumulated
  over jt in PSUM, so W is streamed host-pretransposed (WT[j, i] = W[i, j]).
MLP: gate logits acc[b][o, n] += WxI[k][bf, o].T @ fm[k][bf-slice, n] with
  batch-interleaved host-packed weights WxI (rows = b*192+f), accumulated
  across hops in DRAM via accum_op=add DMAs straight from PSUM.
Chain re-entry: fm -> nm via PE transposes (hops 1,2 of each direction only).
"""

import sys
import numpy as np

for _p in ("/opt/trn_rl_repo",):
    if _p not in sys.path:
        sys.path.insert(0, _p)

from concourse import bacc, tile, mybir  # noqa: E402
from concourse.alu_op_type import AluOpType as ALU  # noqa: E402
from concourse.bass_utils import run_bass_kernel_spmd  # noqa: E402

F32 = mybir.dt.float32
AF = mybir.ActivationFunctionType

C = 4          # batches per core
FI = 192       # per-batch feature width (x 64 + h 128)
BF = C * FI    # 768
DH = 128
NCORES = 8
NHOPS = 3


def build_nc(nt=16):
    """Build + compile the per-core Bass kernel. nt = node tiles (N = nt*128)."""
    N = nt * 128
    nbk = N // 512

    nc = bacc.Bacc("TRN2", target_bir_lowering=False, debug=False,
                   num_devices=NCORES)

    def din(name, shape):
        return nc.dram_tensor(name, shape, F32, kind="ExternalInput").ap()

    XH = din("xh_nm", [nt, 128, BF])
    XHFM = din("xh_fm", [6, 128, N])
    WFT = din("wfT", [nt, 128, N])
    WBT = din("wbT", [nt, 128, N])
    WRI = din("wrI", [7, 6, 128, 128])
    WZI = din("wzI", [7, 6, 128, 128])
    WNI = din("wnI", [7, 6, 128, 128])
    XFM = din("x_fm", [C, 64, N])
    HFM = din("h_fm", [C, 128, N])
    BR = din("br_c", [128, 1])
    BZ = din("bz_c", [128, 1])
    BN = din("bn_c", [128, 1])
    IDT = din("ident", [128, 128])
    OUT = nc.dram_tensor("out_fm", [C, 128, N], F32, kind="ExternalOutput").ap()

    ACCR = nc.dram_tensor("acc_r", [C, 128, N], F32).ap()
    ACCZ = nc.dram_tensor("acc_z", [C, 128, N], F32).ap()
    ACCN = nc.dram_tensor("acc_n", [C, 128, N], F32).ap()
    XRH = nc.dram_tensor("xrh_nm_d", [nt, 128, BF], F32).ap()
    ZD = nc.dram_tensor("z_d", [C, 128, N], F32).ap()

    with tile.TileContext(nc) as tc:
        with (
            tc.tile_pool(name="nm", bufs=36) as nm_pool,
            tc.tile_pool(name="fm", bufs=8) as fm_pool,
            tc.tile_pool(name="wt", bufs=4) as wt_pool,
            tc.tile_pool(name="wxi", bufs=24) as wxi_pool,
            tc.tile_pool(name="stg", bufs=4) as stg_pool,
            tc.tile_pool(name="const", bufs=1) as const_pool,
            tc.tile_pool(name="ps", bufs=8, space="PSUM") as ps_pool,
        ):
            ident = const_pool.tile([128, 128], F32, tag="ident")
            nc.sync.dma_start(ident[:], IDT[:])
            brt = const_pool.tile([128, 1], F32, tag="brt")
            nc.sync.dma_start(brt[:], BR[:])
            bzt = const_pool.tile([128, 1], F32, tag="bzt")
            nc.sync.dma_start(bzt[:], BZ[:])
            bnt = const_pool.tile([128, 1], F32, tag="bnt")
            nc.sync.dma_start(bnt[:], BN[:])

            def load_nm(SRC):
                ts = []
                for jt in range(nt):
                    t = nm_pool.tile([128, BF], F32, name="nmt", tag="nm")
                    nc.sync.dma_start(t[:], SRC[jt])
                    ts.append(t)
                return ts

            def hop(src, WT):
                """One diffusion hop; returns fm tiles (6 x [128, N])."""
                fms = [fm_pool.tile([128, N], F32, name="fmt", tag="fm") for _ in range(6)]
                for ibk in range(nbk):
                    pss = [ps_pool.tile([128, 512], F32, name="pst", tag="ps")
                           for _ in range(6)]
                    for jt in range(nt):
                        wt = wt_pool.tile([128, 512], F32, name="wtt", tag="wt")
                        nc.sync.dma_start(
                            wt[:], WT[jt][:, 512 * ibk:512 * (ibk + 1)])
                        for c in range(6):
                            nc.tensor.matmul(
                                pss[c][:],
                                src[jt][:, 128 * c:128 * (c + 1)],
                                wt[:],
                                start=(jt == 0), stop=(jt == nt - 1))
                    for c in range(6):
                        nc.vector.tensor_copy(
                            fms[c][:, 512 * ibk:512 * (ibk + 1)], pss[c][:])
                return fms

            def mlp_feed(fms, kidx, gates, first):
                """gates: list of (WXI dram, ACC dram). Accumulate logits."""
                for WXI, ACCD in gates:
                    wx = []
                    for t in range(6):
                        w = wxi_pool.tile([128, 128], F32, name="wxit", tag="wxi")
                        nc.gpsimd.dma_start(w[:], WXI[kidx][t])
                        wx.append(w)
                    for b in range(C):
                        # HW constraint: one PSUM accumulation group must not
                        # mix tile_position rows -> split segments by base
                        # partition into two groups and DVE-add them.
                        segs = [divmod(b * FI + 64 * s, 128) for s in range(3)]
                        g0 = [s for s in segs if s[1] == 0]
                        g64 = [s for s in segs if s[1] == 64]
                        for nb in range(nbk):
                            nbs = slice(512 * nb, 512 * (nb + 1))
                            psA = ps_pool.tile([128, 512], F32, name="pst", tag="ps")
                            psB = ps_pool.tile([128, 512], F32, name="pst", tag="ps")
                            for grp, ps in ((g0, psA), (g64, psB)):
                                for i, (t, off) in enumerate(grp):
                                    nc.tensor.matmul(
                                        ps[:],
                                        wx[t][off:off + 64, :],
                                        fms[t][off:off + 64, nbs],
                                        start=(i == 0),
                                        stop=(i == len(grp) - 1))
                            stg = stg_pool.tile([128, 512], F32, name="stgt", tag="stg")
                            nc.vector.tensor_copy(stg[:], psA[:])
                            nc.vector.tensor_add(stg[:], stg[:], psB[:])
                            nc.gpsimd.dma_start(
                                ACCD[b][:, nbs], stg[:],
                                accum_op=(ALU.bypass if first else ALU.add))

            def retranspose(fms):
                """fm tiles -> fresh nm tiles via PE transposes."""
                nms = [nm_pool.tile([128, BF], F32, name="nmt", tag="nm")
                       for _ in range(nt)]
                for it in range(nt):
                    for h2 in range(2):
                        ps = ps_pool.tile([128, 384], F32, name="pst", tag="ps")
                        for q in range(3):
                            c = 3 * h2 + q
                            nc.tensor.transpose(
                                ps[:, 128 * q:128 * (q + 1)],
                                fms[c][:, 128 * it:128 * (it + 1)],
                                ident[:])
                        nc.vector.tensor_copy(
                            nms[it][:, 384 * h2:384 * (h2 + 1)], ps[:])
                return nms

            def diffusion(x_nm_loader, x_fm_tiles, gates, xnm_first=None):
                """Full 2-direction diffusion + MLP accumulation.
                x_nm_loader() -> fresh nm tiles of the chunk-0 tensor.
                x_fm_tiles: fm tiles of chunk 0 (consumed)."""
                mlp_feed(x_fm_tiles, 0, gates, first=True)
                cur = xnm_first if xnm_first is not None else x_nm_loader()
                for k in range(1, NHOPS + 1):
                    fm = hop(cur, WFT)
                    mlp_feed(fm, k, gates, first=False)
                    cur = retranspose(fm) if k < NHOPS else None
                cur = x_nm_loader()
                for k in range(1, NHOPS + 1):
                    fm = hop(cur, WBT)
                    mlp_feed(fm, NHOPS + k, gates, first=False)
                    cur = retranspose(fm) if k < NHOPS else None

            # ---------------- diffusion 1 (r, z gates) ----------------
            fm0 = []
            for t in range(6):
                f = fm_pool.tile([128, N], F32, name="fmt", tag="fm")
                nc.scalar.dma_start(f[:], XHFM[t])
                fm0.append(f)
            diffusion(lambda: load_nm(XH), fm0, [(WRI, ACCR), (WZI, ACCZ)])

            # ------------- gates r, z; assemble xrh (nm + fm) -------------
            xrh_nm = [nm_pool.tile([128, BF], F32, name="nmt", tag="nm")
                      for _ in range(nt)]
            xrh_fm = [fm_pool.tile([128, N], F32, name="fmt", tag="fm") for _ in range(6)]
            for b in range(C):
                accr = fm_pool.tile([128, N], F32, name="fmt", tag="fm")
                nc.scalar.dma_start(accr[:], ACCR[b])
                r = fm_pool.tile([128, N], F32, name="fmt", tag="fm")
                nc.scalar.activation(r[:], accr[:], AF.Sigmoid, bias=brt[:])
                accz = fm_pool.tile([128, N], F32, name="fmt", tag="fm")
                nc.scalar.dma_start(accz[:], ACCZ[b])
                z = fm_pool.tile([128, N], F32, name="fmt", tag="fm")
                nc.scalar.activation(z[:], accz[:], AF.Sigmoid, bias=bzt[:])
                nc.scalar.dma_start(ZD[b], z[:])
                h = fm_pool.tile([128, N], F32, name="fmt", tag="fm")
                nc.scalar.dma_start(h[:], HFM[b])
                rh = fm_pool.tile([128, N], F32, name="fmt", tag="fm")
                nc.vector.tensor_mul(rh[:], r[:], h[:])
                # rh columns of xrh_nm (transpose 128-blocks)
                for g in range(nt // 4):
                    ps = ps_pool.tile([128, 512], F32, name="pst", tag="ps")
                    for q in range(4):
                        it = 4 * g + q
                        nc.tensor.transpose(
                            ps[:, 128 * q:128 * (q + 1)],
                            rh[:, 128 * it:128 * (it + 1)], ident[:])
                    for q in range(4):
                        nc.vector.tensor_copy(
                            xrh_nm[4 * g + q][:, b * FI + 64:(b + 1) * FI],
                            ps[:, 128 * q:128 * (q + 1)])
                # fm rows of xrh: x piece then two rh 64-row pieces
                t, off = divmod(b * FI, 128)
                nc.scalar.dma_start(xrh_fm[t][off:off + 64, :], XFM[b])
                for s2 in range(2):
                    t, off = divmod(b * FI + 64 + 64 * s2, 128)
                    nc.scalar.dma_start(xrh_fm[t][off:off + 64, :],
                                        rh[64 * s2:64 * (s2 + 1), :])
            # x columns of xrh_nm straight from the xh param
            for jt in range(nt):
                for b in range(C):
                    nc.scalar.dma_start(xrh_nm[jt][:, b * FI:b * FI + 64],
                                        XH[jt][:, b * FI:b * FI + 64])
            # spill xrh_nm for the backward-chain reload
            for jt in range(nt):
                nc.sync.dma_start(XRH[jt], xrh_nm[jt][:])

            # ---------------- diffusion 2 (n gate) ----------------
            diffusion(lambda: load_nm(XRH), xrh_fm, [(WNI, ACCN)],
                      xnm_first=xrh_nm)

            # ---------------- final gate ----------------
            for b in range(C):
                accn = fm_pool.tile([128, N], F32, name="fmt", tag="fm")
                nc.scalar.dma_start(accn[:], ACCN[b])
                n_t = fm_pool.tile([128, N], F32, name="fmt", tag="fm")
                nc.scalar.activation(n_t[:], accn[:], AF.Tanh, bias=bnt[:])
                h = fm_pool.tile([128, N], F32, name="fmt", tag="fm")
                nc.scalar.dma_start(h[:], HFM[b])
                z = fm_pool.tile([128, N], F32, name="fmt", tag="fm")
                nc.scalar.dma_start(z[:], ZD[b])
                d = fm_pool.tile([128, N], F32, name="fmt", tag="fm")
                nc.vector.tensor_sub(d[:], n_t[:], h[:])
                zd2 = fm_pool.tile([128, N], F32, name="fmt", tag="fm")
                nc.vector.tensor_mul(zd2[:], z[:], d[:])
                o = fm_pool.tile([128, N], F32, name="fmt", tag="fm")
                nc.vector.tensor_add(o[:], zd2[:], h[:])
                nc.scalar.dma_start(OUT[b], o[:])

    nc.compile()
    return nc


def _pack_interleaved(W):
    """[128, 7*192] torch-Linear weight -> [7, 6, 128, 128] batch-interleaved
    transposed blocks: out[k, t, p, o] = W[o, k*192 + f], row 128t+p = b*192+f."""
    out = np.zeros((7, BF, 128), np.float32)
    for k in range(7):
        blk = np.ascontiguousarray(W[:, k * FI:(k + 1) * FI].T)
        for b in range(C):
            out[k, b * FI:(b + 1) * FI] = blk
    return np.ascontiguousarray(out.reshape(7, 6, 128, 128))


_NC_CACHE = {}


def _get_nc(nt):
    if nt not in _NC_CACHE:
        _NC_CACHE[nt] = build_nc(nt)
    return _NC_CACHE[nt]


def make_in_maps(x, h_prev, W_fwd, W_bwd, Wr, br, Wz, bz, Wn, bn):
    x = np.asarray(x, np.float32)
    h_prev = np.asarray(h_prev, np.float32)
    B, N, Din = x.shape
    nt = N // 128
    WfT = np.ascontiguousarray(np.asarray(W_fwd, np.float32).T).reshape(nt, 128, N)
    WbT = np.ascontiguousarray(np.asarray(W_bwd, np.float32).T).reshape(nt, 128, N)
    wrI = _pack_interleaved(np.asarray(Wr, np.float32))
    wzI = _pack_interleaved(np.asarray(Wz, np.float32))
    wnI = _pack_interleaved(np.asarray(Wn, np.float32))
    ident = np.ascontiguousarray(np.eye(128, dtype=np.float32))
    brc = np.ascontiguousarray(np.asarray(br, np.float32).reshape(128, 1))
    bzc = np.ascontiguousarray(np.asarray(bz, np.float32).reshape(128, 1))
    bnc = np.ascontiguousarray(np.asarray(bn, np.float32).reshape(128, 1))
    ncores = B // C
    in_maps = []
    for cix in range(ncores):
        xs = x[C * cix:C * (cix + 1)]
        hs = h_prev[C * cix:C * (cix + 1)]
        xh = np.concatenate([xs, hs], axis=-1)            # [C, N, 192]
        flat = np.ascontiguousarray(xh.transpose(1, 0, 2).reshape(N, BF))
        xh_nm = np.ascontiguousarray(flat).reshape(nt, 128, BF)
        xh_fm = np.ascontiguousarray(flat.T).reshape(6, 128, N)
        x_fm = np.ascontiguousarray(xs.transpose(0, 2, 1))
        h_fm = np.ascontiguousarray(hs.transpose(0, 2, 1))
        in_maps.append(dict(
            xh_nm=xh_nm, xh_fm=xh_fm, wfT=WfT, wbT=WbT,
            wrI=wrI, wzI=wzI, wnI=wnI, x_fm=x_fm, h_fm=h_fm,
            br_c=brc, bz_c=bzc, bn_c=bnc, ident=ident))
    return in_maps, nt, ncores


def kernel(x, h_prev, W_fwd, W_bwd, Wr, br, Wz, bz, Wn, bn, _trace=False):
    in_maps, nt, ncores = make_in_maps(
        x, h_prev, W_fwd, W_bwd, Wr, br, Wz, bz, Wn, bn)
    nc = _get_nc(nt)
    res = run_bass_kernel_spmd(nc, in_maps, list(range(ncores)), trace=_trace)
    outs = [np.ascontiguousarray(res.results[c]["out_fm"].transpose(0, 2, 1))
            for c in range(ncores)]
    full = np.concatenate(outs, axis=0).astype(np.float32)
    if _trace:
        return full, res
    return full
